# revision 29
# baseline (speedup 1.0000x reference)
"""Distributed (8-core) Trainium2 Bass kernel for nn_Attention.

Reference computation (per batch b of 4, x: [4, 256, 2048]):
  qkv = w_qkv @ x[b]            -> q,k,v each [8 heads, 64, 2048]
  dots = (q^T k) * 64**-0.5     -> [8, 2048, 2048]
  attn = softmax(dots, -1)
  av   = v @ attn^T             -> [8, 64, 2048]
  out  = w_out @ av + b_out     -> [256, 2048]

Sharding: 8 shards = (batch b in 0..3) x (query-half in 0..1). Each core
gets the full x[b] (columns permuted so its own 1024 query positions come
first), computes full k/v (duplicated with its half-partner, ~10% extra
flops, zero communication), q only for its 1024 queries, its half of the
attention, and its half of the final projection. Host concatenates.

On-core dataflow (f32 storage, float32r matmuls, fp32 softmax):
  q[hd,i], k[hd,j] natural; v computed TRANSPOSED [j, hd] via x-stationary
  matmuls. dots computed transposed [j-part, i-free] so AV contracts j on
  partitions. The AV stationary operand is [v_h | ones] ([128, 65]) so the
  softmax denominator accumulates as output partition 64. exp runs on
  ScalarE straight out of PSUM (scale folded), both heads of a pair in one
  [128, 1024] ACTIVATE.
"""

import sys

sys.path.insert(0, "/opt/trn_rl_repo")
sys.path.insert(0, "/root/.axon_site")

import numpy as np

DIM = 256
N = 2048
NQ = 1024
H = 8
DH = 64
HID = 512
PAIRS = 4
SCALE = DH ** -0.5

_CACHE = {}


def _register_ntff_hook():
    """The agent image's antenv lacks axon_hooks; synthesize it so
    run_bass_kernel_spmd(trace=True) can profile. Harmless if unused."""
    import types

    if "antenv.axon_hooks" in sys.modules:
        return
    try:
        import antenv
        from trn_agent_boot.trn_boot import _ntff_profile_via_ctypes

        mod = types.ModuleType("antenv.axon_hooks")
        _hook = [None]
        mod.set_axon_ntff_profile_hook = lambda h: _hook.__setitem__(0, h)
        mod.get_axon_ntff_profile_hook = lambda: _hook[0]
        sys.modules["antenv.axon_hooks"] = mod
        antenv.axon_hooks = mod
        mod.set_axon_ntff_profile_hook(
            _ntff_profile_via_ctypes("/opt/axon/libaxon_pjrt.so")
        )
    except Exception:
        pass


def build_nc():
    import concourse.mybir as mybir
    import concourse.tile as tile
    from concourse import bacc

    f32 = mybir.dt.float32
    bf16 = mybir.dt.bfloat16
    Exp = mybir.ActivationFunctionType.Exp

    nc = bacc.Bacc("TRN2", target_bir_lowering=False, debug=False)

    x_ext = nc.dram_tensor("x", [DIM, N], bf16, kind="ExternalInput")
    wq_ext = nc.dram_tensor("wq_t", [DIM, HID], bf16, kind="ExternalInput")
    wk_ext = nc.dram_tensor("wk_t", [DIM, HID], bf16, kind="ExternalInput")
    wv_ext = nc.dram_tensor("wv_t", [DIM, HID], bf16, kind="ExternalInput")
    wo_ext = nc.dram_tensor("wo_t", [HID, DIM], bf16, kind="ExternalInput")
    b_ext = nc.dram_tensor("bias", [DIM, 1], f32, kind="ExternalInput")
    out_ext = nc.dram_tensor("out", [DIM, NQ], f32, kind="ExternalOutput")

    VSLOT = DH + 1  # 64 v columns + 1 ones column per head

    with tile.TileContext(nc) as tc:
        with (
            tc.tile_pool(name="persist", bufs=1) as pp,
            tc.tile_pool(name="qk", bufs=2) as qk,
            tc.tile_pool(name="epool", bufs=6) as ep,
            tc.tile_pool(name="small", bufs=4) as sp,
            tc.tile_pool(name="pdots", bufs=2, space="PSUM") as pd,
            tc.tile_pool(name="pattn", bufs=3, space="PSUM") as pa,
            tc.tile_pool(name="pproj", bufs=1, space="PSUM") as pj,
        ):
            # ---- warm the ACT exp table early (one tiny op) ----
            dummy = sp.tile([1, 1], f32, tag="dummy")
            nc.vector.memset(dummy[:], 0.0)
            dummy2 = sp.tile([1, 1], f32, tag="dummy2")
            nc.scalar.activation(dummy2[:], dummy[:], Exp)

            # ---- input DMAs (f32) + cast to bf16 ----
            def load_bf16(ext, rows, cols, tag):
                tiles = []
                h = cols // 2
                for cc in range(rows // 128):
                    t = pp.tile([128, cols], bf16, tag=f"{tag}{cc}", name=f"{tag}{cc}")
                    r0, r1 = cc * 128, (cc + 1) * 128
                    nc.sync.dma_start(t[:, 0:h], ext[r0:r1, 0:h])
                    nc.gpsimd.dma_start(t[:, h:cols], ext[r0:r1, h:cols])
                    tiles.append(t)
                return tiles

            # Fine-grained load order: wq + leading x columns first so the
            # first q-projection matmul can start ~9us in; bulk follows.
            wq_sb = [pp.tile([128, HID], bf16, tag=f"wq{c}", name=f"wq{c}") for c in range(2)]
            wk_sb = [pp.tile([128, HID], bf16, tag=f"wk{c}", name=f"wk{c}") for c in range(2)]
            x_sb = [pp.tile([128, N], bf16, tag=f"x{c}", name=f"x{c}") for c in range(2)]
            engs = [nc.sync, nc.gpsimd]
            for c in range(2):
                engs[c].dma_start(wq_sb[c][:], wq_ext[c * 128 : (c + 1) * 128, :])
            for c in range(2):
                engs[c].dma_start(
                    x_sb[c][:, 0:512], x_ext[c * 128 : (c + 1) * 128, 0:512]
                )
            for c in range(2):
                engs[c].dma_start(wk_sb[c][:], wk_ext[c * 128 : (c + 1) * 128, :])
            for c in range(2):
                engs[c].dma_start(
                    x_sb[c][:, 512:1024], x_ext[c * 128 : (c + 1) * 128, 512:1024]
                )
            for c in range(2):
                engs[c].dma_start(
                    x_sb[c][:, 1024:2048], x_ext[c * 128 : (c + 1) * 128, 1024:2048]
                )
            wv_sb = load_bf16(wv_ext, DIM, HID, "wv")
            wo_sb = load_bf16(wo_ext, HID, DIM, "wo")
            bias_sb = pp.tile([128, 2], f32, tag="bias")
            for oc in range(2):
                nc.sync.dma_start(
                    bias_sb[:, oc : oc + 1], b_ext[oc * 128 : (oc + 1) * 128, :]
                )

            # ---- v^T projection: vt[j, hd] for all heads, x chunks stationary ----
            # vt layout per j-chunk: 8 slots of [64 v | 1 ones]
            def qk_proj(p):
                pools = [pd, pd, pj, pj, pj, pj] if p == 0 else [pj] * 6
                tags = (
                    ["dots", "dots", "proj", "proj", "proj", "proj"]
                    if p == 0
                    else ["proj"] * 6
                )
                pi = 0
                q_t = qk.tile([128, NQ], bf16, tag="q", name="q_t")
                k_t = qk.tile([128, N], bf16, tag="k", name="k_t")

                def one(dst_t, w_sb, col):
                    nonlocal pi
                    ps = pools[pi].tile([128, 512], f32, tag=tags[pi], name="ps")
                    pi += 1
                    for cc in range(2):
                        nc.tensor.matmul(
                            ps[:],
                            lhsT=w_sb[cc][:, p * 128 : (p + 1) * 128],
                            rhs=x_sb[cc][:, col : col + 512],
                            start=(cc == 0),
                            stop=(cc == 1),
                        )
                    nc.vector.tensor_copy(dst_t[:, col : col + 512], ps[:])

                one(q_t, wq_sb, 0)      # queries for ic0
                one(k_t, wk_sb, 0)      # keys j 0:512 (first 4 jc)
                one(q_t, wq_sb, 512)    # queries for ic1
                for jc4 in range(1, 4):
                    one(k_t, wk_sb, jc4 * 512)
                return q_t, k_t

            qk0 = qk_proj(0)

            ones_sb = pp.tile([128, H], f32, tag="ones")
            nc.vector.memset(ones_sb[:], 1.0)
            vt = pp.tile([128, 16 * H * VSLOT], bf16, tag="vt")
            for jc in range(16):
                ps = pj.tile([128, HID], f32, tag="proj", name="ps")
                for cc in range(2):
                    nc.tensor.matmul(
                        ps[:],
                        lhsT=x_sb[cc][:, jc * 128 : (jc + 1) * 128],
                        rhs=wv_sb[cc][:],
                        start=(cc == 0),
                        stop=(cc == 1),
                    )
                vslice = vt[
                    :, jc * H * VSLOT : (jc + 1) * H * VSLOT
                ].rearrange("p (h s) -> p h s", s=VSLOT)
                nc.vector.tensor_copy(
                    vslice[:, :, 0:DH],
                    ps[:].rearrange("p (h d) -> p h d", d=DH),
                )
                nc.vector.tensor_copy(
                    vslice[:, :, DH : DH + 1],
                    ones_sb[:].rearrange("p (h o) -> p h o", o=1),
                )

            attn_n = [
                pp.tile([128, NQ], bf16, tag=f"attn_n{p}", name=f"attn_n{p}")
                for p in range(PAIRS)
            ]

            # ---- per head-pair: q/k projection then attention ----
            out_acc = [
                pp.tile([128, NQ], f32, tag=f"oacc{oc}", name=f"oacc{oc}")
                for oc in range(2)
            ]
            qk_next = qk0
            for p in range(PAIRS):
                q_t, k_t = qk_next

                def norm_phase1(attA, attB):
                    # copy accumulators out of PSUM (frees the banks fast)
                    out = []
                    for att in (attA, attB):
                        att_s = sp.tile([65, 512], f32, tag="att_s", name="att_s")
                        nc.vector.tensor_copy(att_s[:], att[0:65, :])
                        den = sp.tile([1, 512], f32, tag="den")
                        nc.vector.tensor_copy(den[:], att_s[64:65, :])
                        out.append((att_s, den))
                    return out

                def norm_phase2(ic, staged):
                    # reciprocal + broadcast + normalize into attn_n
                    for hh, (att_s, den) in enumerate(staged):
                        rec = sp.tile([1, 512], f32, tag="rec")
                        nc.vector.reciprocal_approx_fast(rec[:], den[:])
                        recb = sp.tile([64, 512], f32, tag="recb")
                        nc.gpsimd.partition_broadcast(recb[:], rec[:])
                        # issued twice: Q7 completion can signal before its
                        # SBUF writes retire; the duplicate (idempotent) op
                        # makes the consumer wait one full op longer.
                        nc.gpsimd.partition_broadcast(recb[:], rec[:])
                        nc.vector.tensor_mul(
                            attn_n[p][
                                hh * 64 : (hh + 1) * 64, ic * 512 : (ic + 1) * 512
                            ],
                            att_s[0:64, :],
                            recb[:],
                        )

                pending = None
                for ic in range(2):
                    if ic == 1 and p < PAIRS - 1:
                        # emit next pair's projections here so their PSUM-pool
                        # slots precede this pair's out-proj tiles (lets the
                        # scheduler hoist them into this pair's attention)
                        qk_next = qk_proj(p + 1)
                    if ic == 1 and pending is not None:
                        # phase-2 of ic0's normalization AFTER the next pair's
                        # projection CASTs, so the bcast-gated muls don't
                        # head-of-line-block the DVE queue
                        norm_phase2(0, pending)
                        pending = None
                    attA = pa.tile([128, 512], f32, tag="att", name="attA")
                    attB = pa.tile([128, 512], f32, tag="att", name="attB")
                    for jc in range(16):
                        d = pd.tile([128, 1024], f32, tag="dots", name="d")
                        # head A = 2p (k rows 0:64), head B = 2p+1 (rows 64:128)
                        nc.tensor.matmul(
                            d[:, 0:512],
                            lhsT=k_t[0:64, jc * 128 : (jc + 1) * 128],
                            rhs=q_t[0:64, ic * 512 : (ic + 1) * 512],
                            start=True,
                            stop=True,
                        )
                        nc.tensor.matmul(
                            d[:, 512:1024],
                            lhsT=k_t[64:128, jc * 128 : (jc + 1) * 128],
                            rhs=q_t[64:128, ic * 512 : (ic + 1) * 512],
                            start=True,
                            stop=True,
                        )
                        e = ep.tile([128, 1024], bf16, tag="e")
                        nc.scalar.activation(e[:], d[:], Exp, scale=SCALE)
                        base = jc * H * VSLOT
                        hA = 2 * p
                        hB = 2 * p + 1
                        nc.tensor.matmul(
                            attA[0 : DH + 1, :],
                            lhsT=vt[:, base + hA * VSLOT : base + (hA + 1) * VSLOT],
                            rhs=e[:, 0:512],
                            start=(jc == 0),
                            stop=(jc == 15),
                        )
                        nc.tensor.matmul(
                            attB[0 : DH + 1, :],
                            lhsT=vt[:, base + hB * VSLOT : base + (hB + 1) * VSLOT],
                            rhs=e[:, 512:1024],
                            start=(jc == 0),
                            stop=(jc == 15),
                        )
                    staged = norm_phase1(attA, attB)
                    if ic == 0:
                        pending = staged
                    else:
                        norm_phase2(1, staged)

                # partial output projection for this pair, accumulated in SBUF
                for ic2 in range(2):
                    for oc in range(2):
                        ps = pj.tile([128, 512], f32, tag="proj", name="ops")
                        nc.tensor.matmul(
                            ps[:],
                            lhsT=wo_sb[p][:, oc * 128 : (oc + 1) * 128],
                            rhs=attn_n[p][:, ic2 * 512 : (ic2 + 1) * 512],
                            start=True,
                            stop=True,
                        )
                        dst = out_acc[oc][:, ic2 * 512 : (ic2 + 1) * 512]
                        if p == 0:
                            nc.vector.tensor_scalar_add(
                                dst, ps[:], bias_sb[:, oc : oc + 1]
                            )
                        else:
                            nc.vector.tensor_add(dst, dst, ps[:])
                        if p == PAIRS - 1:
                            nc.sync.dma_start(
                                out_ext[
                                    oc * 128 : (oc + 1) * 128,
                                    ic2 * 512 : (ic2 + 1) * 512,
                                ],
                                dst,
                            )

    nc.compile()
    return nc


def _shard_inputs(x, w_qkv, w_out, b_out):
    """Returns in_maps for cores 0..7; core c = (batch c//2, query-half c%2)."""
    x = np.asarray(x, dtype=np.float32)
    w_qkv = np.asarray(w_qkv, dtype=np.float32)
    w_out = np.asarray(w_out, dtype=np.float32)
    b_out = np.asarray(b_out, dtype=np.float32)

    import ml_dtypes

    bf = ml_dtypes.bfloat16
    wq_t = np.ascontiguousarray(w_qkv[0:HID].T).astype(bf)  # [256, 512]
    wk_t = np.ascontiguousarray(w_qkv[HID : 2 * HID].T).astype(bf)
    wv_t = np.ascontiguousarray(w_qkv[2 * HID : 3 * HID].T).astype(bf)
    wo_t = np.ascontiguousarray(w_out.T).astype(bf)  # [512, 256]
    bias = np.ascontiguousarray(b_out.reshape(DIM, 1))

    in_maps = []
    for c in range(8):
        b, half = divmod(c, 2)
        xb = x[b]
        halves = [xb[:, 0:NQ], xb[:, NQ:N]]
        x_perm = np.ascontiguousarray(
            np.concatenate([halves[half], halves[1 - half]], axis=1)
        ).astype(ml_dtypes.bfloat16)
        in_maps.append(
            {
                "x": x_perm,
                "wq_t": wq_t,
                "wk_t": wk_t,
                "wv_t": wv_t,
                "wo_t": wo_t,
                "bias": bias,
            }
        )
    return in_maps


def run(x, w_qkv, w_out, b_out, trace=False, tmpdir=None):
    from concourse.bass_utils import run_bass_kernel_spmd

    _register_ntff_hook()
    if "nc" not in _CACHE:
        _CACHE["nc"] = build_nc()
    nc = _CACHE["nc"]
    in_maps = _shard_inputs(x, w_qkv, w_out, b_out)
    kw = {}
    if trace:
        kw.update(trace=True, tmpdir=tmpdir)
    res = run_bass_kernel_spmd(nc, in_maps, core_ids=list(range(8)), **kw)
    out = np.empty((4, DIM, N), dtype=np.float32)
    for c in range(8):
        b, half = divmod(c, 2)
        out[b][:, half * NQ : (half + 1) * NQ] = res.results[c]["out"]
    return out, res


def kernel(**inputs):
    out, _ = run(
        inputs["x"], inputs["w_qkv"], inputs["w_out"], inputs["b_out"]
    )
    return out


# revision 30
# speedup vs baseline: 1.0324x; 1.0324x over previous
"""Distributed (8-core) Trainium2 Bass kernel for nn_Attention.

Reference computation (per batch b of 4, x: [4, 256, 2048]):
  qkv = w_qkv @ x[b]            -> q,k,v each [8 heads, 64, 2048]
  dots = (q^T k) * 64**-0.5     -> [8, 2048, 2048]
  attn = softmax(dots, -1)
  av   = v @ attn^T             -> [8, 64, 2048]
  out  = w_out @ av + b_out     -> [256, 2048]

Sharding: 8 shards = (batch b in 0..3) x (query-half in 0..1). Each core
gets the full x[b] (columns permuted so its own 1024 query positions come
first), computes full k/v (duplicated with its half-partner, ~10% extra
flops, zero communication), q only for its 1024 queries, its half of the
attention, and its half of the final projection. Host concatenates.

On-core dataflow (f32 storage, float32r matmuls, fp32 softmax):
  q[hd,i], k[hd,j] natural; v computed TRANSPOSED [j, hd] via x-stationary
  matmuls. dots computed transposed [j-part, i-free] so AV contracts j on
  partitions. The AV stationary operand is [v_h | ones] ([128, 65]) so the
  softmax denominator accumulates as output partition 64. exp runs on
  ScalarE straight out of PSUM (scale folded), both heads of a pair in one
  [128, 1024] ACTIVATE.
"""

import sys

sys.path.insert(0, "/opt/trn_rl_repo")
sys.path.insert(0, "/root/.axon_site")

import numpy as np

DIM = 256
N = 2048
NQ = 1024
H = 8
DH = 64
HID = 512
PAIRS = 4
SCALE = DH ** -0.5

_CACHE = {}


def _register_ntff_hook():
    """The agent image's antenv lacks axon_hooks; synthesize it so
    run_bass_kernel_spmd(trace=True) can profile. Harmless if unused."""
    import types

    if "antenv.axon_hooks" in sys.modules:
        return
    try:
        import antenv
        from trn_agent_boot.trn_boot import _ntff_profile_via_ctypes

        mod = types.ModuleType("antenv.axon_hooks")
        _hook = [None]
        mod.set_axon_ntff_profile_hook = lambda h: _hook.__setitem__(0, h)
        mod.get_axon_ntff_profile_hook = lambda: _hook[0]
        sys.modules["antenv.axon_hooks"] = mod
        antenv.axon_hooks = mod
        mod.set_axon_ntff_profile_hook(
            _ntff_profile_via_ctypes("/opt/axon/libaxon_pjrt.so")
        )
    except Exception:
        pass


def build_nc():
    import concourse.mybir as mybir
    import concourse.tile as tile
    from concourse import bacc

    f32 = mybir.dt.float32
    bf16 = mybir.dt.bfloat16
    Exp = mybir.ActivationFunctionType.Exp

    nc = bacc.Bacc("TRN2", target_bir_lowering=False, debug=False)

    x_ext = nc.dram_tensor("x", [DIM, N], bf16, kind="ExternalInput")
    wq_ext = nc.dram_tensor("wq_t", [DIM, HID], bf16, kind="ExternalInput")
    wk_ext = nc.dram_tensor("wk_t", [DIM, HID], bf16, kind="ExternalInput")
    wv_ext = nc.dram_tensor("wv_t", [DIM, HID], bf16, kind="ExternalInput")
    wo_ext = nc.dram_tensor("wo_t", [HID, DIM], bf16, kind="ExternalInput")
    b_ext = nc.dram_tensor("bias", [DIM, 1], f32, kind="ExternalInput")
    out_ext = nc.dram_tensor("out", [DIM, NQ], f32, kind="ExternalOutput")

    VSLOT = DH + 1  # 64 v columns + 1 ones column per head

    with tile.TileContext(nc) as tc:
        with (
            tc.tile_pool(name="persist", bufs=1) as pp,
            tc.tile_pool(name="qk", bufs=2) as qk,
            tc.tile_pool(name="epool", bufs=6) as ep,
            tc.tile_pool(name="small", bufs=4) as sp,
            tc.tile_pool(name="pdots", bufs=2, space="PSUM") as pd,
            tc.tile_pool(name="pattn", bufs=3, space="PSUM") as pa,
            tc.tile_pool(name="pproj", bufs=1, space="PSUM") as pj,
        ):
            # ---- warm the ACT exp table early (one tiny op) ----
            dummy = sp.tile([1, 1], f32, tag="dummy")
            nc.vector.memset(dummy[:], 0.0)
            dummy2 = sp.tile([1, 1], f32, tag="dummy2")
            nc.scalar.activation(dummy2[:], dummy[:], Exp)

            # ---- input DMAs (f32) + cast to bf16 ----
            def load_bf16(ext, rows, cols, tag):
                tiles = []
                h = cols // 2
                for cc in range(rows // 128):
                    t = pp.tile([128, cols], bf16, tag=f"{tag}{cc}", name=f"{tag}{cc}")
                    r0, r1 = cc * 128, (cc + 1) * 128
                    nc.sync.dma_start(t[:, 0:h], ext[r0:r1, 0:h])
                    nc.gpsimd.dma_start(t[:, h:cols], ext[r0:r1, h:cols])
                    tiles.append(t)
                return tiles

            # Fine-grained load order: wq + leading x columns first so the
            # first q-projection matmul can start ~9us in; bulk follows.
            wq_sb = [pp.tile([128, HID], bf16, tag=f"wq{c}", name=f"wq{c}") for c in range(2)]
            wk_sb = [pp.tile([128, HID], bf16, tag=f"wk{c}", name=f"wk{c}") for c in range(2)]
            x_sb = [pp.tile([128, N], bf16, tag=f"x{c}", name=f"x{c}") for c in range(2)]
            engs = [nc.sync, nc.gpsimd]
            for c in range(2):
                engs[c].dma_start(wq_sb[c][:], wq_ext[c * 128 : (c + 1) * 128, :])
            for c in range(2):
                engs[c].dma_start(
                    x_sb[c][:, 0:512], x_ext[c * 128 : (c + 1) * 128, 0:512]
                )
            for c in range(2):
                engs[c].dma_start(wk_sb[c][:], wk_ext[c * 128 : (c + 1) * 128, :])
            for c in range(2):
                engs[c].dma_start(
                    x_sb[c][:, 512:1024], x_ext[c * 128 : (c + 1) * 128, 512:1024]
                )
            for c in range(2):
                engs[c].dma_start(
                    x_sb[c][:, 1024:2048], x_ext[c * 128 : (c + 1) * 128, 1024:2048]
                )
            wv_sb = load_bf16(wv_ext, DIM, HID, "wv")
            wo_sb = load_bf16(wo_ext, HID, DIM, "wo")
            bias_sb = pp.tile([128, 2], f32, tag="bias")
            for oc in range(2):
                nc.sync.dma_start(
                    bias_sb[:, oc : oc + 1], b_ext[oc * 128 : (oc + 1) * 128, :]
                )

            # ---- v^T projection: vt[j, hd] for all heads, x chunks stationary ----
            # vt layout per j-chunk: 8 slots of [64 v | 1 ones]
            def qk_proj(p):
                pools = [pd, pd, pj, pj, pj, pj] if p == 0 else [pj] * 6
                tags = (
                    ["dots", "dots", "proj", "proj", "proj", "proj"]
                    if p == 0
                    else ["proj"] * 6
                )
                pi = 0
                q_t = qk.tile([128, NQ], bf16, tag="q", name="q_t")
                k_t = qk.tile([128, N], bf16, tag="k", name="k_t")

                def one(dst_t, w_sb, col):
                    nonlocal pi
                    ps = pools[pi].tile([128, 512], f32, tag=tags[pi], name="ps")
                    pi += 1
                    for cc in range(2):
                        nc.tensor.matmul(
                            ps[:],
                            lhsT=w_sb[cc][:, p * 128 : (p + 1) * 128],
                            rhs=x_sb[cc][:, col : col + 512],
                            start=(cc == 0),
                            stop=(cc == 1),
                        )
                    nc.vector.tensor_copy(dst_t[:, col : col + 512], ps[:])

                one(q_t, wq_sb, 0)      # queries for ic0
                one(k_t, wk_sb, 0)      # keys j 0:512 (first 4 jc)
                one(q_t, wq_sb, 512)    # queries for ic1
                for jc4 in range(1, 4):
                    one(k_t, wk_sb, jc4 * 512)
                return q_t, k_t

            qk0 = qk_proj(0)

            ones_sb = pp.tile([128, H], f32, tag="ones")
            nc.vector.memset(ones_sb[:], 1.0)
            vt = pp.tile([128, 16 * H * VSLOT], bf16, tag="vt")
            for jc in range(16):
                ps = pj.tile([128, HID], f32, tag="proj", name="ps")
                for cc in range(2):
                    nc.tensor.matmul(
                        ps[:],
                        lhsT=x_sb[cc][:, jc * 128 : (jc + 1) * 128],
                        rhs=wv_sb[cc][:],
                        start=(cc == 0),
                        stop=(cc == 1),
                    )
                vslice = vt[
                    :, jc * H * VSLOT : (jc + 1) * H * VSLOT
                ].rearrange("p (h s) -> p h s", s=VSLOT)
                nc.vector.tensor_copy(
                    vslice[:, :, 0:DH],
                    ps[:].rearrange("p (h d) -> p h d", d=DH),
                )
                nc.vector.tensor_copy(
                    vslice[:, :, DH : DH + 1],
                    ones_sb[:].rearrange("p (h o) -> p h o", o=1),
                )

            attn_n = [
                pp.tile([128, NQ], bf16, tag=f"attn_n{p}", name=f"attn_n{p}")
                for p in range(PAIRS)
            ]

            # ---- per head-pair: q/k projection then attention ----
            out_acc = [
                pp.tile([128, NQ], f32, tag=f"oacc{oc}", name=f"oacc{oc}")
                for oc in range(2)
            ]
            qk_next = qk0
            for p in range(PAIRS):
                q_t, k_t = qk_next

                for ic in range(2):
                    if ic == 1 and p < PAIRS - 1:
                        # emit next pair's projections here so their PSUM-pool
                        # slots precede this pair's out-proj tiles (lets the
                        # scheduler hoist them into this pair's attention)
                        qk_next = qk_proj(p + 1)
                    attA = pa.tile([128, 512], f32, tag="att", name="attA")
                    attB = pa.tile([128, 512], f32, tag="att", name="attB")
                    for jc in range(16):
                        d = pd.tile([128, 1024], f32, tag="dots", name="d")
                        # head A = 2p (k rows 0:64), head B = 2p+1 (rows 64:128)
                        nc.tensor.matmul(
                            d[:, 0:512],
                            lhsT=k_t[0:64, jc * 128 : (jc + 1) * 128],
                            rhs=q_t[0:64, ic * 512 : (ic + 1) * 512],
                            start=True,
                            stop=True,
                        )
                        nc.tensor.matmul(
                            d[:, 512:1024],
                            lhsT=k_t[64:128, jc * 128 : (jc + 1) * 128],
                            rhs=q_t[64:128, ic * 512 : (ic + 1) * 512],
                            start=True,
                            stop=True,
                        )
                        e = ep.tile([128, 1024], bf16, tag="e")
                        nc.scalar.activation(e[:], d[:], Exp, scale=SCALE)
                        base = jc * H * VSLOT
                        hA = 2 * p
                        hB = 2 * p + 1
                        nc.tensor.matmul(
                            attA[0 : DH + 1, :],
                            lhsT=vt[:, base + hA * VSLOT : base + (hA + 1) * VSLOT],
                            rhs=e[:, 0:512],
                            start=(jc == 0),
                            stop=(jc == 15),
                        )
                        nc.tensor.matmul(
                            attB[0 : DH + 1, :],
                            lhsT=vt[:, base + hB * VSLOT : base + (hB + 1) * VSLOT],
                            rhs=e[:, 512:1024],
                            start=(jc == 0),
                            stop=(jc == 15),
                        )
                    # normalize: rows 0:64 / row 64, into attn_n[p].
                    # First copy the raw [65,512] to SBUF so the PSUM slot
                    # frees after one DVE op instead of the whole chain.
                    for hh, att in ((0, attA), (1, attB)):
                        # one copy frees the PSUM slot; den comes from SBUF
                        att_s = sp.tile([65, 512], f32, tag="att_s", name="att_s")
                        nc.vector.tensor_copy(att_s[:], att[0:65, :])
                        den = sp.tile([1, 512], f32, tag="den")
                        nc.vector.tensor_copy(den[:], att_s[64:65, :])
                        rec = sp.tile([1, 512], f32, tag="rec")
                        nc.vector.reciprocal_approx_fast(rec[:], den[:])
                        recb = sp.tile([64, 512], f32, tag="recb")
                        nc.gpsimd.partition_broadcast(recb[:], rec[:])
                        # issued twice: Q7 completion can signal before its
                        # SBUF writes retire; the duplicate (idempotent) op
                        # makes the consumer wait one full op longer, and a
                        # race against the 2nd op's writes reads identical
                        # already-retired values from the 1st.
                        nc.gpsimd.partition_broadcast(recb[:], rec[:])
                        nc.vector.tensor_mul(
                            attn_n[p][hh * 64 : (hh + 1) * 64, ic * 512 : (ic + 1) * 512],
                            att_s[0:64, :],
                            recb[:],
                        )

                # partial output projection for this pair, accumulated in SBUF
                for ic2 in range(2):
                    for oc in range(2):
                        ps = pj.tile([128, 512], f32, tag="proj", name="ops")
                        nc.tensor.matmul(
                            ps[:],
                            lhsT=wo_sb[p][:, oc * 128 : (oc + 1) * 128],
                            rhs=attn_n[p][:, ic2 * 512 : (ic2 + 1) * 512],
                            start=True,
                            stop=True,
                        )
                        dst = out_acc[oc][:, ic2 * 512 : (ic2 + 1) * 512]
                        if p == 0:
                            nc.vector.tensor_scalar_add(
                                dst, ps[:], bias_sb[:, oc : oc + 1]
                            )
                        else:
                            nc.vector.tensor_add(dst, dst, ps[:])
                        if p == PAIRS - 1:
                            nc.sync.dma_start(
                                out_ext[
                                    oc * 128 : (oc + 1) * 128,
                                    ic2 * 512 : (ic2 + 1) * 512,
                                ],
                                dst,
                            )

    nc.compile()
    return nc


def _shard_inputs(x, w_qkv, w_out, b_out):
    """Returns in_maps for cores 0..7; core c = (batch c//2, query-half c%2)."""
    x = np.asarray(x, dtype=np.float32)
    w_qkv = np.asarray(w_qkv, dtype=np.float32)
    w_out = np.asarray(w_out, dtype=np.float32)
    b_out = np.asarray(b_out, dtype=np.float32)

    import ml_dtypes

    bf = ml_dtypes.bfloat16
    wq_t = np.ascontiguousarray(w_qkv[0:HID].T).astype(bf)  # [256, 512]
    wk_t = np.ascontiguousarray(w_qkv[HID : 2 * HID].T).astype(bf)
    wv_t = np.ascontiguousarray(w_qkv[2 * HID : 3 * HID].T).astype(bf)
    wo_t = np.ascontiguousarray(w_out.T).astype(bf)  # [512, 256]
    bias = np.ascontiguousarray(b_out.reshape(DIM, 1))

    in_maps = []
    for c in range(8):
        b, half = divmod(c, 2)
        xb = x[b]
        halves = [xb[:, 0:NQ], xb[:, NQ:N]]
        x_perm = np.ascontiguousarray(
            np.concatenate([halves[half], halves[1 - half]], axis=1)
        ).astype(ml_dtypes.bfloat16)
        in_maps.append(
            {
                "x": x_perm,
                "wq_t": wq_t,
                "wk_t": wk_t,
                "wv_t": wv_t,
                "wo_t": wo_t,
                "bias": bias,
            }
        )
    return in_maps


def run(x, w_qkv, w_out, b_out, trace=False, tmpdir=None):
    from concourse.bass_utils import run_bass_kernel_spmd

    _register_ntff_hook()
    if "nc" not in _CACHE:
        _CACHE["nc"] = build_nc()
    nc = _CACHE["nc"]
    in_maps = _shard_inputs(x, w_qkv, w_out, b_out)
    kw = {}
    if trace:
        kw.update(trace=True, tmpdir=tmpdir)
    res = run_bass_kernel_spmd(nc, in_maps, core_ids=list(range(8)), **kw)
    out = np.empty((4, DIM, N), dtype=np.float32)
    for c in range(8):
        b, half = divmod(c, 2)
        out[b][:, half * NQ : (half + 1) * NQ] = res.results[c]["out"]
    return out, res


def kernel(**inputs):
    out, _ = run(
        inputs["x"], inputs["w_qkv"], inputs["w_out"], inputs["b_out"]
    )
    return out
